# revision 2
# baseline (speedup 1.0000x reference)
"""Trainium2 Bass kernel for nn_CSBrain (per-region electrode conv, kernels 1/3/5).

Strategy:
  - Data-parallel over batch: 8 cores x 2 batches each.
  - Host marshals x into an f-major (transposed) fp16 layout with a per-region
    circular halo (2 electrodes each side) and an appended ones-row, so the
    bias can ride the matmul as an extra contraction row.
  - Weights are host-packed into a single (region, 201, 500) fp16 "Wcat":
    columns grouped by output-electrode offset delta in {+2,+1,0,-1,-2} so each
    (electrode, f-half) stationary tile needs only <=5 matmuls, each writing a
    contiguous column range of the per-electrode PSUM accumulator.
  - Device: per (batch, region): load x tiles, for each extended electrode slot
    run matmuls with the x tile stationary (lhsT) and Wcat columns moving,
    accumulating out[t, d] tiles in PSUM (fp32); drain pairs of finished
    electrodes through DVE/ACT copies (cast fp16) and DMA out.
  - Host unscrambles the (b, t, c, d) fp16 device output to (B, C, T, D) fp32.
"""

import sys

if "/opt/trn_rl_repo" not in sys.path:
    sys.path.insert(0, "/opt/trn_rl_repo")

import numpy as np

REGION_SIZES = [12, 14, 12, 14, 12]
REGION_STARTS = [0, 12, 26, 38, 52]
B, C, T, F = 16, 64, 128, 200
DIM_OUT = 200
N_CORES = 8
B_LOC = B // N_CORES  # 2
HALO = 2
SLOTS = [ne + 2 * HALO for ne in REGION_SIZES]  # 16,18,16,18,16
COL_OFFS = np.cumsum([0] + [s * T for s in SLOTS]).tolist()  # per-region col offset
NCOLS = COL_OFFS[-1]  # 84*128 = 10752
KLO = 128  # f rows 0:128 in the lo tile
KHI = F - KLO + 1  # 73 = f rows 128:200 plus the ones/bias row

# Wcat column ranges per delta group (delta = out_electrode - in_electrode)
GCOLS = {2: (0, 50), 1: (50, 150), 0: (150, 350), -1: (350, 450), -2: (450, 500)}
# matching output column ranges in the per-electrode accumulator
OCOLS = {2: (150, 200), 1: (100, 200), 0: (0, 200), -1: (100, 200), -2: (150, 200)}
DELTAS = (2, 1, 0, -1, -2)

_CACHE = {}


def _build_nc():
    import concourse.tile as tile
    from concourse import bacc, mybir
    import concourse.bass as bass

    f16 = mybir.dt.float16
    f32 = mybir.dt.float32

    nc = bacc.Bacc(
        "TRN2",
        target_bir_lowering=False,
        debug=False,
        num_devices=N_CORES,
    )
    xin = nc.dram_tensor("xin", [B_LOC, 201, NCOLS], f16, kind="ExternalInput").ap()
    wcat = nc.dram_tensor("wcat", [5, 201, 500], f16, kind="ExternalInput").ap()
    out = nc.dram_tensor(
        "out", [B_LOC, T, C * DIM_OUT], f16, kind="ExternalOutput"
    ).ap()

    with tile.TileContext(nc) as tc:
        with (
            tc.tile_pool(name="w", bufs=1) as wpool,
            tc.tile_pool(name="x", bufs=2) as xpool,
            tc.tile_pool(name="ps", bufs=8, space=bass.MemorySpace.PSUM) as pspool,
            tc.tile_pool(name="st", bufs=4) as stpool,
        ):
            wlo, whi = [], []
            for r in range(5):
                tl = wpool.tile([KLO, 500], f16, tag=f"wlo{r}")
                nc.sync.dma_start(tl[:], wcat[r, 0:KLO, :])
                th = wpool.tile([KHI, 500], f16, tag=f"whi{r}")
                nc.sync.dma_start(th[:], wcat[r, KLO : KLO + KHI, :])
                wlo.append(tl)
                whi.append(th)

            for bl in range(B_LOC):
                for r in range(5):
                    ne = REGION_SIZES[r]
                    slots = SLOTS[r]
                    off = COL_OFFS[r]
                    ncols = slots * T
                    XL = xpool.tile([KLO, ncols], f16, tag="xl")
                    nc.sync.dma_start(XL[:], xin[bl, 0:KLO, off : off + ncols])
                    XH = xpool.tile([KHI, ncols], f16, tag="xh")
                    nc.sync.dma_start(
                        XH[:], xin[bl, KLO : KLO + KHI, off : off + ncols]
                    )
                    acc = {}
                    for s in range(slots):
                        for half in (0, 1):
                            if half == 0:
                                xt = XL[:, s * T : (s + 1) * T]
                                w = wlo[r]
                            else:
                                xt = XH[:, s * T : (s + 1) * T]
                                w = whi[r]
                            for delta in DELTAS:
                                e = s - HALO + delta
                                if not (0 <= e < ne):
                                    continue
                                if half == 0 and delta == 2:
                                    acc[e] = pspool.tile([T, DIM_OUT], f32, tag="acc", name="acc")
                                g0, g1 = GCOLS[delta]
                                o0, o1 = OCOLS[delta]
                                nc.tensor.matmul(
                                    acc[e][:, o0:o1],
                                    xt,
                                    w[:, g0:g1],
                                    start=(half == 0 and delta == 2),
                                    stop=(half == 1 and delta == -2),
                                )
                        edone = s - 2 * HALO
                        if 0 <= edone < ne and edone % 2 == 1:
                            stage = stpool.tile([T, 2 * DIM_OUT], f16, tag="stage")
                            nc.vector.tensor_copy(
                                stage[:, 0:DIM_OUT], acc[edone - 1][:]
                            )
                            nc.scalar.copy(
                                stage[:, DIM_OUT : 2 * DIM_OUT], acc[edone][:]
                            )
                            cabs = REGION_STARTS[r] + edone - 1
                            nc.sync.dma_start(
                                out[
                                    bl,
                                    :,
                                    cabs * DIM_OUT : (cabs + 2) * DIM_OUT,
                                ],
                                stage[:],
                            )
                            del acc[edone - 1], acc[edone]

    nc.compile()
    return nc


def _get_nc():
    if "nc" not in _CACHE:
        _CACHE["nc"] = _build_nc()
    return _CACHE["nc"]


def _marshal_x(x):
    """x (B, C, T, F) fp32 -> (N_CORES, B_LOC, 201, NCOLS) fp16, f-major with
    halo and ones-row."""
    xin = np.empty((B, 201, NCOLS), np.float16)
    for r in range(5):
        ne = REGION_SIZES[r]
        s0 = REGION_STARTS[r]
        off = COL_OFFS[r]
        idx = (np.arange(SLOTS[r]) - HALO) % ne
        xr = x[:, s0 + idx, :, :]  # (B, S, T, F)
        arr = np.transpose(xr, (0, 3, 1, 2)).reshape(B, F, SLOTS[r] * T)
        xin[:, 0:F, off : off + SLOTS[r] * T] = arr.astype(np.float16)
    xin[:, F, :] = np.float16(1.0)
    return xin.reshape(N_CORES, B_LOC, 201, NCOLS)


def _marshal_w(W1, b1, W3, b3, W5, b5):
    """Pack weights into (5, 201, 500) fp16 Wcat (f rows 0:200, bias row 200)."""
    wcat = np.zeros((5, 201, 500), np.float32)

    def put(col, W, j):
        d = W.shape[1]
        wcat[:, 0:F, col : col + d] = np.transpose(W[:, :, :, j], (0, 2, 1))
        return col + d

    # delta=+2 : k5 j0
    put(0, W5, 0)
    # delta=+1 : k3 j0, k5 j1
    put(50, W3, 0)
    put(100, W5, 1)
    # delta=0 : k1 j0, k3 j1, k5 j2 (center taps -> carry bias)
    put(150, W1, 0)
    put(250, W3, 1)
    put(300, W5, 2)
    wcat[:, F, 150:250] = b1
    wcat[:, F, 250:300] = b3
    wcat[:, F, 300:350] = b5
    # delta=-1 : k3 j2, k5 j3
    put(350, W3, 2)
    put(400, W5, 3)
    # delta=-2 : k5 j4
    put(450, W5, 4)
    return wcat.astype(np.float16)


def _unmarshal(outs):
    """outs: list of N_CORES arrays (B_LOC, T, C*DIM_OUT) fp16 -> (B,C,T,D) fp32."""
    dev = np.stack(outs).reshape(B, T, C, DIM_OUT)
    return np.ascontiguousarray(dev.transpose(0, 2, 1, 3)).astype(np.float32)


def _run(in_maps, **kwargs):
    from concourse.bass_utils import run_bass_kernel_spmd

    nc = _get_nc()
    return run_bass_kernel_spmd(nc, in_maps, core_ids=list(range(N_CORES)), **kwargs)


def make_in_maps(x, W1, b1, W3, b3, W5, b5):
    xin = _marshal_x(np.asarray(x, dtype=np.float32))
    wcat = _marshal_w(
        np.asarray(W1), np.asarray(b1), np.asarray(W3), np.asarray(b3),
        np.asarray(W5), np.asarray(b5),
    )
    return [{"xin": xin[m], "wcat": wcat} for m in range(N_CORES)]


def kernel(x, W1, b1, W3, b3, W5, b5):
    in_maps = make_in_maps(x, W1, b1, W3, b3, W5, b5)
    res = _run(in_maps)
    return _unmarshal([res.results[m]["out"] for m in range(N_CORES)])
